# revision 14
# baseline (speedup 1.0000x reference)
"""Trainium2 Bass kernel for nn_MeshEdgeBlock (GNN edge-block message passing).

Computes, per edge e with endpoints (s, d):
    x  = concat([src_nodes[s], dst_nodes[d], edge_feat[e]])   # [384]
    h  = silu(x @ W1 + b1)                                    # [512]
    y  = h @ W2 + b2                                          # [128]
    y  = LayerNorm(y) * gamma + beta + edge_feat[e]           # [128]

Sharding: edges (and index arrays) split evenly across the 8 NeuronCores;
node-feature tables and MLP weights replicated to every core.

Device-side dataflow per core (EC = 31360 padded edges, 245 tiles of 128
edges, grouped G=7 tiles per "supertile"):
  - one indirect-DMA gather per supertile pulls src+dst node rows (bf16)
    from a host-concatenated [2N, 128] table (dst indices offset by N)
  - per 128-edge tile, DMA(xbar)-transpose the three 128x128 bf16 feature
    blocks into xT (features-on-partitions) for the matmuls
  - mm1: 12 bf16 128^3 matmuls -> hT psum bank [128h, 4 chunks x 128 edges]
  - silu on ScalarE in one pass over the bank (b1 == 0 fast path)
  - mm2: 4 bf16 matmuls -> y[128 edges, 128] psum (fp32)
  - LN stats per tile via bn_stats/bn_aggr (VectorE); y copied to SBUF by
    ScalarE (Copy - same activation table set as Silu, so no table reloads)
  - rsqrt(var+eps) computed once per supertile on VectorE with the
    exponent-bit seed + 2 Newton steps (no Sqrt on ScalarE: sqrt lives in a
    different activation-function table set and each switch reloads tables)
  - normalize + residual fused into one custom-DVE affine_then_add:
    out = (y * inv + (-mu*inv)) + edge_feat

Numerics: matmul inputs bf16 (fp32 PSUM accumulation); LN + residual fp32
except edge features, which stay bf16 end-to-end. b1/b2/gamma/beta get a
fast path when they hold the trivial values hardcoded by the problem's
setup_inputs (zeros/ones) - verified on the host per call; non-trivial
values take extra (exact, slightly slower) ops.
"""

import numpy as np
import ml_dtypes
from contextlib import ExitStack

import concourse.bass as bass
import concourse.tile as tile
from concourse import bacc, mybir
from concourse.bass import IndirectOffsetOnAxis
from concourse.bass_utils import run_bass_kernel_spmd
from concourse.masks import make_identity

# Problem constants (hardcoded per spec)
N_CORES = 8
E_FULL = 250000
N_NODES = 100000
D = 128          # node/edge feature dim == LN dim
H = 512          # hidden dim
LN_EPS = 1e-5

G = 7            # 128-edge tiles per supertile
EC = 31360       # padded edges per core; EC = 245*128, 245 = 35*G
NT = EC // 128   # 245 edge tiles per core
NS = NT // G     # 35 supertiles per core

BF16 = mybir.dt.bfloat16
F32 = mybir.dt.float32
I32 = mybir.dt.int32

RSQRT_MAGIC = 0x5F3759DF

# transpose path: 'dma' = xbar DMA transpose (HWDGE), 'pe' = TensorE+identity
TP_MODE = "pe"

_PROGRAM_CACHE = {}


def _rsqrt_batched(nc, stats, mg2, eps_negh):
    """inv = rsqrt(var + eps), nmi = -mu * inv, batched over the supertile.

    mg2: [128, 2G] f32 with (mean, var) pairs per tile; eps_negh unused slot.
    Returns (inv, nmi) [128, G] tiles. Uses the fp32 exponent-bit seed plus
    two Newton steps y <- y*(1.5 + (-veps/2)*y^2); rel err ~5e-6, plenty
    inside LN given bf16 matmul inputs.
    """
    mu = mg2[:, 0:2 * G:2]
    var = mg2[:, 1:2 * G:2]
    veps = stats.tile([128, G], F32, tag="veps")
    nc.vector.tensor_scalar(out=veps[:], in0=var, scalar1=LN_EPS, scalar2=None,
                            op0=mybir.AluOpType.add)
    hv = stats.tile([128, G], F32, tag="hv")
    nc.vector.tensor_scalar(out=hv[:], in0=veps[:], scalar1=-0.5, scalar2=None,
                            op0=mybir.AluOpType.mult)
    sh = stats.tile([128, G], I32, tag="sh")
    nc.vector.tensor_scalar(out=sh[:], in0=veps[:].bitcast(I32), scalar1=1,
                            scalar2=None,
                            op0=mybir.AluOpType.arith_shift_right)
    seed = stats.tile([128, G], I32, tag="seed")
    nc.vector.tensor_scalar(out=seed[:], in0=sh[:], scalar1=-1,
                            scalar2=RSQRT_MAGIC,
                            op0=mybir.AluOpType.mult,
                            op1=mybir.AluOpType.add)
    y = seed[:].bitcast(F32)
    for it in range(2):
        a = stats.tile([128, G], F32, tag=f"nr_a{it}")
        nc.vector.tensor_mul(out=a[:], in0=y, in1=y)
        b = stats.tile([128, G], F32, tag=f"nr_b{it}")
        nc.vector.tensor_mul(out=b[:], in0=a[:], in1=hv[:])
        ynew = stats.tile([128, G], F32, tag=f"nr_y{it}")
        nc.vector.scalar_tensor_tensor(out=ynew[:], in0=b[:], scalar=1.5,
                                       in1=y, op0=mybir.AluOpType.add,
                                       op1=mybir.AluOpType.mult)
        y = ynew[:]
    nmi = stats.tile([128, G], F32, tag="nmi")
    nc.vector.scalar_tensor_tensor(out=nmi[:], in0=mu, scalar=-1.0, in1=y,
                                   op0=mybir.AluOpType.mult,
                                   op1=mybir.AluOpType.mult)
    return y, nmi


def _build_program(trivial_affine: bool, sim_safe: bool = False,
                   repeats: int = 1):
    """Build (and cache) the Bass program. Returns the compiled Bacc.

    sim_safe=True replaces the Silu activation (not implemented in CoreSim)
    with Sigmoid + an explicit multiply; used only for simulator validation.
    repeats>1 wraps the whole body in a hardware For loop (benchmarking).
    """
    key = (trivial_affine, sim_safe, TP_MODE, repeats)
    if key in _PROGRAM_CACHE:
        return _PROGRAM_CACHE[key]

    nc = bacc.Bacc("TRN2", target_bir_lowering=False, debug=False,
                   num_devices=N_CORES)

    nodes = nc.dram_tensor("nodes", [2 * N_NODES, D], BF16, kind="ExternalInput").ap()
    edges = nc.dram_tensor("edges", [EC, D], BF16, kind="ExternalInput").ap()
    idx = nc.dram_tensor("idx", [EC, 2], I32, kind="ExternalInput").ap()
    w1 = nc.dram_tensor("w1", [D, 12 * D], BF16, kind="ExternalInput").ap()
    w2 = nc.dram_tensor("w2", [D, 4 * D], BF16, kind="ExternalInput").ap()
    out = nc.dram_tensor("out", [EC, D], F32, kind="ExternalOutput").ap()
    if not trivial_affine:
        b1d = nc.dram_tensor("b1d", [D, 4], F32, kind="ExternalInput").ap()
        b2d = nc.dram_tensor("b2d", [D, D], F32, kind="ExternalInput").ap()
        gmd = nc.dram_tensor("gmd", [D, D], F32, kind="ExternalInput").ap()
        btd = nc.dram_tensor("btd", [D, D], F32, kind="ExternalInput").ap()

    with tile.TileContext(nc) as tc, ExitStack() as ctx:
        const = ctx.enter_context(tc.tile_pool(name="const", bufs=1))
        io = ctx.enter_context(tc.tile_pool(name="io", bufs=2))
        idxp = ctx.enter_context(tc.tile_pool(name="idx", bufs=2))
        xtp = ctx.enter_context(tc.tile_pool(name="xt", bufs=3))
        htp = ctx.enter_context(tc.tile_pool(name="ht", bufs=3))
        stats = ctx.enter_context(tc.tile_pool(name="stats", bufs=2))
        ps_ht = ctx.enter_context(tc.tile_pool(name="ps_ht", bufs=3, space="PSUM"))
        ps_y = ctx.enter_context(tc.tile_pool(name="ps_y", bufs=3, space="PSUM"))
        if TP_MODE == "pe":
            ps_tp = ctx.enter_context(tc.tile_pool(name="ps_tp", bufs=2, space="PSUM"))

        # constants
        w1sb = const.tile([D, 12 * D], BF16)
        nc.sync.dma_start(out=w1sb[:], in_=w1[:])
        w2sb = const.tile([D, 4 * D], BF16)
        nc.sync.dma_start(out=w2sb[:], in_=w2[:])
        if TP_MODE == "pe":
            ident = const.tile([D, D], BF16)
            make_identity(nc, ident[:])
        if not trivial_affine:
            b1sb = const.tile([D, 4], F32)
            nc.sync.dma_start(out=b1sb[:], in_=b1d[:])
            b2sb = const.tile([D, D], F32)
            nc.sync.dma_start(out=b2sb[:], in_=b2d[:])
            gmsb = const.tile([D, D], F32)
            nc.sync.dma_start(out=gmsb[:], in_=gmd[:])
            btsb = const.tile([D, D], F32)
            nc.sync.dma_start(out=btsb[:], in_=btd[:])

        def _supertile(t):
            base = t * G * 128
            # combined src/dst indices: [p, 2g] = src edge(p,g), [p, 2g+1] = dst
            it_ = idxp.tile([128, 2 * G], I32, tag="idx")
            nc.sync.dma_start(
                out=it_[:],
                in_=idx[base:base + G * 128, :].rearrange("(p g) c -> p (g c)", g=G))
            sd = io.tile([128, 2 * G, D], BF16, tag="sd")
            # one [128,1]-index gather per column: walrus's indirect-DMA
            # lowering only agrees with the sim for one gathered row per
            # partition (multi-column offset APs fetch the wrong rows on HW)
            for k in range(2 * G):
                nc.gpsimd.indirect_dma_start(
                    out=sd[:, k, :], out_offset=None, in_=nodes[:],
                    in_offset=IndirectOffsetOnAxis(ap=it_[:, k:k + 1], axis=0))
            e_bf = io.tile([128, G, D], BF16, tag="e")
            nc.sync.dma_start(
                out=e_bf[:],
                in_=edges[base:base + G * 128, :].rearrange("(p g) f -> p g f", g=G))
            ysb = io.tile([128, G, D], F32, tag="ysb")
            yout = io.tile([128, G, D], F32, tag="yout")
            mg2 = stats.tile([128, 2 * G], F32, tag="mg2")

            for g in range(G):
                xt = xtp.tile([128, 3 * D], BF16)
                if TP_MODE == "dma":
                    nc.sync.dma_start(out=xt[:, 0:D], in_=sd[:, 2 * g, :], transpose=True)
                    nc.sync.dma_start(out=xt[:, D:2 * D], in_=sd[:, 2 * g + 1, :], transpose=True)
                    nc.sync.dma_start(out=xt[:, 2 * D:3 * D], in_=e_bf[:, g, :], transpose=True)
                else:
                    tp = ps_tp.tile([128, 3 * D], BF16)
                    nc.tensor.transpose(out=tp[:, 0:D], in_=sd[:, 2 * g, :], identity=ident[:])
                    nc.tensor.transpose(out=tp[:, D:2 * D], in_=sd[:, 2 * g + 1, :], identity=ident[:])
                    nc.tensor.transpose(out=tp[:, 2 * D:3 * D], in_=e_bf[:, g, :], identity=ident[:])
                    nc.vector.tensor_copy(out=xt[:], in_=tp[:])

                # mm1: hT[m-chunk partitions, edge free] for 4 chunks
                htps = ps_ht.tile([128, H], F32)
                for m in range(4):
                    for c in range(3):
                        nc.tensor.matmul(
                            out=htps[:, m * D:(m + 1) * D],
                            lhsT=w1sb[:, (c * 4 + m) * D:(c * 4 + m + 1) * D],
                            rhs=xt[:, c * D:(c + 1) * D],
                            start=(c == 0), stop=(c == 2))

                ht = htp.tile([128, H], BF16)
                if sim_safe:
                    sg = htp.tile([128, H], BF16, tag="sg")
                    nc.scalar.activation(out=sg[:], in_=htps[:],
                                         func=mybir.ActivationFunctionType.Sigmoid)
                    nc.vector.tensor_mul(out=ht[:], in0=htps[:], in1=sg[:])
                    if not trivial_affine:
                        raise NotImplementedError("sim_safe assumes trivial affine")
                elif trivial_affine:
                    nc.scalar.activation(out=ht[:], in_=htps[:],
                                         func=mybir.ActivationFunctionType.Silu)
                else:
                    for m in range(4):
                        nc.scalar.activation(
                            out=ht[:, m * D:(m + 1) * D],
                            in_=htps[:, m * D:(m + 1) * D],
                            func=mybir.ActivationFunctionType.Silu,
                            bias=b1sb[:, m:m + 1])

                # mm2: y[edge partitions, feature free]
                yps = ps_y.tile([128, D], F32)
                for m in range(4):
                    nc.tensor.matmul(
                        out=yps[:],
                        lhsT=ht[:, m * D:(m + 1) * D],
                        rhs=w2sb[:, m * D:(m + 1) * D],
                        start=(m == 0), stop=(m == 3))

                if not trivial_affine:
                    # y += b2 (pre-LN); write combined into ysb
                    nc.vector.tensor_add(out=ysb[:, g, :], in0=yps[:], in1=b2sb[:])
                else:
                    nc.scalar.activation(out=ysb[:, g, :], in_=yps[:],
                                         func=mybir.ActivationFunctionType.Copy)

                st6 = stats.tile([128, 6], F32, tag="st6")
                nc.vector.bn_stats(out=st6[:], in_=ysb[:, g, :])
                nc.vector.bn_aggr(out=mg2[:, 2 * g:2 * g + 2], in_=st6[:])

            inv, nmi = _rsqrt_batched(nc, stats, mg2, None)
            for g in range(G):
                if trivial_affine:
                    nc.vector.affine_then_add(
                        out=yout[:, g, :], in0=ysb[:, g, :], in1=e_bf[:, g, :],
                        scale=inv[:, g:g + 1], bias=nmi[:, g:g + 1])
                else:
                    yn = io.tile([128, D], F32, tag="yn")
                    nc.vector.tensor_scalar(out=yn[:], in0=ysb[:, g, :],
                                            scalar1=inv[:, g:g + 1],
                                            scalar2=nmi[:, g:g + 1],
                                            op0=mybir.AluOpType.mult,
                                            op1=mybir.AluOpType.add)
                    nc.vector.tensor_mul(out=yn[:], in0=yn[:], in1=gmsb[:])
                    nc.vector.tensor_add(out=yn[:], in0=yn[:], in1=btsb[:])
                    nc.vector.tensor_add(out=yout[:, g, :], in0=yn[:], in1=e_bf[:, g, :])

            nc.sync.dma_start(
                out=out[base:base + G * 128, :].rearrange("(p g) f -> p g f", g=G),
                in_=yout[:])

        if repeats == 1:
            for t in range(NS):
                _supertile(t)
        else:
            with tc.For_i(0, repeats, 1):
                for t in range(NS):
                    _supertile(t)

    nc.compile()
    _PROGRAM_CACHE[key] = nc
    return nc


def _prep(inputs):
    """Host-side preprocessing -> per-core input maps + metadata."""
    f = {k: np.asarray(v) for k, v in inputs.items()}
    bf = ml_dtypes.bfloat16

    nodes = np.concatenate([f["src_node_features"], f["dst_node_features"]],
                           axis=0).astype(bf)

    e = f["edge_features"].astype(np.float32)
    si = f["src_indices"].astype(np.int64)
    di = f["dst_indices"].astype(np.int64)
    E = e.shape[0]
    etot = EC * N_CORES
    e_pad = np.zeros((etot, D), np.float32)
    e_pad[:E] = e
    idx_pad = np.zeros((etot, 2), np.int32)
    idx_pad[:E, 0] = si.astype(np.int32)
    idx_pad[:E, 1] = (di + N_NODES).astype(np.int32)
    idx_pad[E:, 1] = N_NODES  # padding rows gather row 0 of each half
    e_bf = e_pad.astype(bf)

    W1 = f["W1"].astype(np.float32)
    W2 = f["W2"].astype(np.float32)
    w1b = np.concatenate(
        [W1[c * D:(c + 1) * D, m * D:(m + 1) * D] for c in range(3) for m in range(4)],
        axis=1).astype(bf)
    w2b = np.concatenate([W2[m * D:(m + 1) * D, :] for m in range(4)], axis=1).astype(bf)

    b1 = f["b1"].astype(np.float32)
    b2 = f["b2"].astype(np.float32)
    gm = f["ln_gamma"].astype(np.float32)
    bt = f["ln_beta"].astype(np.float32)
    trivial = (not b1.any()) and (not b2.any()) and (not bt.any()) and bool(np.all(gm == 1.0))

    in_maps = []
    for i in range(N_CORES):
        lo, hi = i * EC, (i + 1) * EC
        m = {
            "nodes": nodes,
            "edges": np.ascontiguousarray(e_bf[lo:hi]),
            "idx": np.ascontiguousarray(idx_pad[lo:hi]),
            "w1": w1b,
            "w2": w2b,
        }
        if not trivial:
            m["b1d"] = np.ascontiguousarray(b1.reshape(4, D).T.astype(np.float32))
            m["b2d"] = np.broadcast_to(b2, (D, D)).copy()
            m["gmd"] = np.broadcast_to(gm, (D, D)).copy()
            m["btd"] = np.broadcast_to(bt, (D, D)).copy()
        in_maps.append(m)
    return in_maps, trivial, E


def kernel(**inputs) -> np.ndarray:
    in_maps, trivial, E = _prep(inputs)
    nc = _build_program(trivial)
    res = run_bass_kernel_spmd(nc, in_maps, core_ids=list(range(N_CORES)))
    out = np.concatenate([res.results[i]["out"] for i in range(N_CORES)], axis=0)
    return np.ascontiguousarray(out[:E])


# revision 24
# speedup vs baseline: 1.0080x; 1.0080x over previous
"""Trainium2 Bass kernel for nn_MeshEdgeBlock (GNN edge-block message passing).

Computes, per edge e with endpoints (s, d):
    x  = concat([src_nodes[s], dst_nodes[d], edge_feat[e]])   # [384]
    h  = silu(x @ W1 + b1)                                    # [512]
    y  = h @ W2 + b2                                          # [128]
    y  = LayerNorm(y) * gamma + beta + edge_feat[e]           # [128]

Sharding: edges (and index arrays) split evenly across the 8 NeuronCores;
node-feature tables and MLP weights replicated to every core.

Device-side dataflow per core (EC = 31360 padded edges, 245 tiles of 128
edges, grouped G=7 tiles per "supertile"):
  - one indirect-DMA gather per supertile pulls src+dst node rows (bf16)
    from a host-concatenated [2N, 128] table (dst indices offset by N)
  - per 128-edge tile, DMA(xbar)-transpose the three 128x128 bf16 feature
    blocks into xT (features-on-partitions) for the matmuls
  - mm1: 12 bf16 128^3 matmuls -> hT psum bank [128h, 4 chunks x 128 edges]
  - silu on ScalarE in one pass over the bank (b1 == 0 fast path)
  - mm2: 4 bf16 matmuls -> y[128 edges, 128] psum (fp32)
  - LN stats per tile via bn_stats/bn_aggr (VectorE); y copied to SBUF by
    ScalarE (Copy - same activation table set as Silu, so no table reloads)
  - rsqrt(var+eps) computed once per supertile on VectorE with the
    exponent-bit seed + 2 Newton steps (no Sqrt on ScalarE: sqrt lives in a
    different activation-function table set and each switch reloads tables)
  - normalize + residual fused into one custom-DVE affine_then_add:
    out = (y * inv + (-mu*inv)) + edge_feat

Numerics: matmul inputs bf16 (fp32 PSUM accumulation); LN + residual fp32
except edge features, which stay bf16 end-to-end. b1/b2/gamma/beta get a
fast path when they hold the trivial values hardcoded by the problem's
setup_inputs (zeros/ones) - verified on the host per call; non-trivial
values take extra (exact, slightly slower) ops.
"""

import numpy as np
import ml_dtypes
from contextlib import ExitStack

import concourse.bass as bass
import concourse.tile as tile
from concourse import bacc, mybir
from concourse.bass import IndirectOffsetOnAxis
from concourse.bass_utils import run_bass_kernel_spmd
from concourse.masks import make_identity

# Problem constants (hardcoded per spec)
N_CORES = 8
E_FULL = 250000
N_NODES = 100000
D = 128          # node/edge feature dim == LN dim
H = 512          # hidden dim
LN_EPS = 1e-5

G = 7            # 128-edge tiles per supertile
EC = 31360       # padded edges per core; EC = 245*128, 245 = 35*G
NT = EC // 128   # 245 edge tiles per core
NS = NT // G     # 35 supertiles per core

BF16 = mybir.dt.bfloat16
F32 = mybir.dt.float32
I32 = mybir.dt.int32

RSQRT_MAGIC = 0x5F3759DF

# transpose path: 'dma' = xbar DMA transpose (HWDGE), 'pe' = TensorE+identity
TP_MODE = "pe"

# bench bisection: 'full' | 'nogather' (skip indirect DMAs) | 'gatheronly'
# (skip transposes/matmuls/LN) | 'dmaonly' (skip gathers and compute)
VARIANT = "full"

_PROGRAM_CACHE = {}


def _rsqrt_batched(nc, stats, mg2, eps_negh):
    """inv = rsqrt(var + eps), nmi = -mu * inv, batched over the supertile.

    mg2: [128, 2G] f32 with (mean, var) pairs per tile; eps_negh unused slot.
    Returns (inv, nmi) [128, G] tiles. Uses the fp32 exponent-bit seed plus
    two Newton steps y <- y*(1.5 + (-veps/2)*y^2); rel err ~5e-6, plenty
    inside LN given bf16 matmul inputs.
    """
    mu = mg2[:, 0:2 * G:2]
    var = mg2[:, 1:2 * G:2]
    veps = stats.tile([128, G], F32, tag="veps")
    nc.vector.tensor_scalar(out=veps[:], in0=var, scalar1=LN_EPS, scalar2=None,
                            op0=mybir.AluOpType.add)
    hv = stats.tile([128, G], F32, tag="hv")
    nc.vector.tensor_scalar(out=hv[:], in0=veps[:], scalar1=-0.5, scalar2=None,
                            op0=mybir.AluOpType.mult)
    sh = stats.tile([128, G], I32, tag="sh")
    nc.vector.tensor_scalar(out=sh[:], in0=veps[:].bitcast(I32), scalar1=1,
                            scalar2=None,
                            op0=mybir.AluOpType.arith_shift_right)
    seed = stats.tile([128, G], I32, tag="seed")
    nc.vector.tensor_scalar(out=seed[:], in0=sh[:], scalar1=-1,
                            scalar2=RSQRT_MAGIC,
                            op0=mybir.AluOpType.mult,
                            op1=mybir.AluOpType.add)
    y = seed[:].bitcast(F32)
    for it in range(2):
        a = stats.tile([128, G], F32, tag=f"nr_a{it}")
        nc.vector.tensor_mul(out=a[:], in0=y, in1=y)
        b = stats.tile([128, G], F32, tag=f"nr_b{it}")
        nc.vector.tensor_mul(out=b[:], in0=a[:], in1=hv[:])
        ynew = stats.tile([128, G], F32, tag=f"nr_y{it}")
        nc.vector.scalar_tensor_tensor(out=ynew[:], in0=b[:], scalar=1.5,
                                       in1=y, op0=mybir.AluOpType.add,
                                       op1=mybir.AluOpType.mult)
        y = ynew[:]
    nmi = stats.tile([128, G], F32, tag="nmi")
    nc.vector.scalar_tensor_tensor(out=nmi[:], in0=mu, scalar=-1.0, in1=y,
                                   op0=mybir.AluOpType.mult,
                                   op1=mybir.AluOpType.mult)
    return y, nmi


def _build_program(trivial_affine: bool, sim_safe: bool = False,
                   repeats: int = 1):
    """Build (and cache) the Bass program. Returns the compiled Bacc.

    sim_safe=True replaces the Silu activation (not implemented in CoreSim)
    with Sigmoid + an explicit multiply; used only for simulator validation.
    repeats>1 wraps the whole body in a hardware For loop (benchmarking).
    """
    key = (trivial_affine, sim_safe, TP_MODE, repeats, VARIANT)
    if key in _PROGRAM_CACHE:
        return _PROGRAM_CACHE[key]
    do_gather = VARIANT in ("full", "gatheronly")
    do_compute = VARIANT in ("full", "nogather")

    nc = bacc.Bacc("TRN2", target_bir_lowering=False, debug=False,
                   num_devices=N_CORES)

    nodes = nc.dram_tensor("nodes", [2 * N_NODES, D], BF16, kind="ExternalInput").ap()
    edges = nc.dram_tensor("edges", [EC, D], BF16, kind="ExternalInput").ap()
    idx = nc.dram_tensor("idx", [EC, 2], I32, kind="ExternalInput").ap()
    w1 = nc.dram_tensor("w1", [D, 12 * D], BF16, kind="ExternalInput").ap()
    w2 = nc.dram_tensor("w2", [D, 4 * D], BF16, kind="ExternalInput").ap()
    out = nc.dram_tensor("out", [EC, D], F32, kind="ExternalOutput").ap()
    scratch = None
    if VARIANT in ("gatheronly", "dmaonly"):
        scratch = nc.dram_tensor("scratch", [2 * G * 128, D], BF16).ap()
    if not trivial_affine:
        b1d = nc.dram_tensor("b1d", [D, 4], F32, kind="ExternalInput").ap()
        b2d = nc.dram_tensor("b2d", [D, D], F32, kind="ExternalInput").ap()
        gmd = nc.dram_tensor("gmd", [D, D], F32, kind="ExternalInput").ap()
        btd = nc.dram_tensor("btd", [D, D], F32, kind="ExternalInput").ap()

    with tile.TileContext(nc) as tc, ExitStack() as ctx:
        const = ctx.enter_context(tc.tile_pool(name="const", bufs=1))
        io = ctx.enter_context(tc.tile_pool(name="io", bufs=3))
        idxp = ctx.enter_context(tc.tile_pool(name="idx", bufs=4))
        xtp = ctx.enter_context(tc.tile_pool(name="xt", bufs=3))
        htp = ctx.enter_context(tc.tile_pool(name="ht", bufs=3))
        stats = ctx.enter_context(tc.tile_pool(name="stats", bufs=2))
        ps_ht = ctx.enter_context(tc.tile_pool(name="ps_ht", bufs=3, space="PSUM"))
        ps_y = ctx.enter_context(tc.tile_pool(name="ps_y", bufs=3, space="PSUM"))
        if TP_MODE == "pe":
            ps_tp = ctx.enter_context(tc.tile_pool(name="ps_tp", bufs=2, space="PSUM"))

        # constants
        w1sb = const.tile([D, 12 * D], BF16)
        nc.sync.dma_start(out=w1sb[:], in_=w1[:])
        w2sb = const.tile([D, 4 * D], BF16)
        nc.sync.dma_start(out=w2sb[:], in_=w2[:])
        if TP_MODE == "pe":
            ident = const.tile([D, D], BF16)
            make_identity(nc, ident[:])
        if not trivial_affine:
            b1sb = const.tile([D, 4], F32)
            nc.sync.dma_start(out=b1sb[:], in_=b1d[:])
            b2sb = const.tile([D, D], F32)
            nc.sync.dma_start(out=b2sb[:], in_=b2d[:])
            gmsb = const.tile([D, D], F32)
            nc.sync.dma_start(out=gmsb[:], in_=gmd[:])
            btsb = const.tile([D, D], F32)
            nc.sync.dma_start(out=btsb[:], in_=btd[:])

        def _supertile(t):
            base = t * G * 128
            # combined src/dst indices: [p, 2g] = src edge(p,g), [p, 2g+1] = dst
            it_ = idxp.tile([128, 2 * G], I32, tag="idx")
            nc.sync.dma_start(
                out=it_[:],
                in_=idx[base:base + G * 128, :].rearrange("(p g) c -> p (g c)", g=G))
            sd = io.tile([128, 2 * G, D], BF16, tag="sd")
            # one [128,1]-index gather per column: walrus's indirect-DMA
            # lowering only agrees with the sim for one gathered row per
            # partition (multi-column offset APs fetch the wrong rows on HW)
            if do_gather:
                for k in range(2 * G):
                    nc.gpsimd.indirect_dma_start(
                        out=sd[:, k, :], out_offset=None, in_=nodes[:],
                        in_offset=IndirectOffsetOnAxis(ap=it_[:, k:k + 1], axis=0))
            else:
                # bisect probe: same bytes, plain contiguous DMA
                b2 = (t * 2 * G * 128) % (2 * N_NODES - 2 * G * 128)
                nc.sync.dma_start(
                    out=sd[:],
                    in_=nodes[b2:b2 + 2 * G * 128, :].rearrange(
                        "(p k) f -> p k f", k=2 * G))
            e_bf = io.tile([128, G, D], BF16, tag="e")
            nc.sync.dma_start(
                out=e_bf[:],
                in_=edges[base:base + G * 128, :].rearrange("(p g) f -> p g f", g=G))
            if do_compute:
                ysb = io.tile([128, G, D], F32, tag="ysb")
                yout = io.tile([128, G, D], F32, tag="yout")
                mg2 = stats.tile([128, 2 * G], F32, tag="mg2")

            for g in range(G if do_compute else 0):
                xt = xtp.tile([128, 3 * D], BF16)
                if TP_MODE == "dma":
                    nc.sync.dma_start(out=xt[:, 0:D], in_=sd[:, 2 * g, :], transpose=True)
                    nc.sync.dma_start(out=xt[:, D:2 * D], in_=sd[:, 2 * g + 1, :], transpose=True)
                    nc.sync.dma_start(out=xt[:, 2 * D:3 * D], in_=e_bf[:, g, :], transpose=True)
                else:
                    tp = ps_tp.tile([128, 3 * D], BF16)
                    nc.tensor.transpose(out=tp[:, 0:D], in_=sd[:, 2 * g, :], identity=ident[:])
                    nc.tensor.transpose(out=tp[:, D:2 * D], in_=sd[:, 2 * g + 1, :], identity=ident[:])
                    nc.tensor.transpose(out=tp[:, 2 * D:3 * D], in_=e_bf[:, g, :], identity=ident[:])
                    nc.vector.tensor_copy(out=xt[:], in_=tp[:])

                # mm1: hT[m-chunk partitions, edge free] for 4 chunks
                htps = ps_ht.tile([128, H], F32)
                for m in range(4):
                    for c in range(3):
                        nc.tensor.matmul(
                            out=htps[:, m * D:(m + 1) * D],
                            lhsT=w1sb[:, (c * 4 + m) * D:(c * 4 + m + 1) * D],
                            rhs=xt[:, c * D:(c + 1) * D],
                            start=(c == 0), stop=(c == 2))

                ht = htp.tile([128, H], BF16)
                if sim_safe:
                    sg = htp.tile([128, H], BF16, tag="sg")
                    nc.scalar.activation(out=sg[:], in_=htps[:],
                                         func=mybir.ActivationFunctionType.Sigmoid)
                    nc.vector.tensor_mul(out=ht[:], in0=htps[:], in1=sg[:])
                    if not trivial_affine:
                        raise NotImplementedError("sim_safe assumes trivial affine")
                elif trivial_affine:
                    nc.scalar.activation(out=ht[:], in_=htps[:],
                                         func=mybir.ActivationFunctionType.Silu)
                else:
                    for m in range(4):
                        nc.scalar.activation(
                            out=ht[:, m * D:(m + 1) * D],
                            in_=htps[:, m * D:(m + 1) * D],
                            func=mybir.ActivationFunctionType.Silu,
                            bias=b1sb[:, m:m + 1])

                # mm2: y[edge partitions, feature free]
                yps = ps_y.tile([128, D], F32)
                for m in range(4):
                    nc.tensor.matmul(
                        out=yps[:],
                        lhsT=ht[:, m * D:(m + 1) * D],
                        rhs=w2sb[:, m * D:(m + 1) * D],
                        start=(m == 0), stop=(m == 3))

                if not trivial_affine:
                    # y += b2 (pre-LN); write combined into ysb
                    nc.vector.tensor_add(out=ysb[:, g, :], in0=yps[:], in1=b2sb[:])
                else:
                    nc.scalar.activation(out=ysb[:, g, :], in_=yps[:],
                                         func=mybir.ActivationFunctionType.Copy)

                st6 = stats.tile([128, 6], F32, tag="st6")
                nc.vector.bn_stats(out=st6[:], in_=ysb[:, g, :])
                nc.vector.bn_aggr(out=mg2[:, 2 * g:2 * g + 2], in_=st6[:])

            if do_compute:
                inv, nmi = _rsqrt_batched(nc, stats, mg2, None)
            for g in range(G if do_compute else 0):
                if trivial_affine:
                    nc.vector.affine_then_add(
                        out=yout[:, g, :], in0=ysb[:, g, :], in1=e_bf[:, g, :],
                        scale=inv[:, g:g + 1], bias=nmi[:, g:g + 1])
                else:
                    yn = io.tile([128, D], F32, tag="yn")
                    nc.vector.tensor_scalar(out=yn[:], in0=ysb[:, g, :],
                                            scalar1=inv[:, g:g + 1],
                                            scalar2=nmi[:, g:g + 1],
                                            op0=mybir.AluOpType.mult,
                                            op1=mybir.AluOpType.add)
                    nc.vector.tensor_mul(out=yn[:], in0=yn[:], in1=gmsb[:])
                    nc.vector.tensor_add(out=yn[:], in0=yn[:], in1=btsb[:])
                    nc.vector.tensor_add(out=yout[:, g, :], in0=yn[:], in1=e_bf[:, g, :])

            if do_compute:
                nc.sync.dma_start(
                    out=out[base:base + G * 128, :].rearrange("(p g) f -> p g f", g=G),
                    in_=yout[:])
            else:
                nc.sync.dma_start(
                    out=scratch[:].rearrange("(p k) f -> p k f", k=2 * G),
                    in_=sd[:])

        if repeats == 1:
            for t in range(NS):
                _supertile(t)
        else:
            with tc.For_i(0, repeats, 1):
                for t in range(NS):
                    _supertile(t)

    nc.compile()
    _PROGRAM_CACHE[key] = nc
    return nc


def _prep(inputs):
    """Host-side preprocessing -> per-core input maps + metadata."""
    f = {k: np.asarray(v) for k, v in inputs.items()}
    bf = ml_dtypes.bfloat16

    nodes = np.concatenate([f["src_node_features"], f["dst_node_features"]],
                           axis=0).astype(bf)

    e = f["edge_features"].astype(np.float32)
    si = f["src_indices"].astype(np.int64)
    di = f["dst_indices"].astype(np.int64)
    E = e.shape[0]
    etot = EC * N_CORES
    e_pad = np.zeros((etot, D), np.float32)
    e_pad[:E] = e
    idx_pad = np.zeros((etot, 2), np.int32)
    idx_pad[:E, 0] = si.astype(np.int32)
    idx_pad[:E, 1] = (di + N_NODES).astype(np.int32)
    idx_pad[E:, 1] = N_NODES  # padding rows gather row 0 of each half
    e_bf = e_pad.astype(bf)

    W1 = f["W1"].astype(np.float32)
    W2 = f["W2"].astype(np.float32)
    w1b = np.concatenate(
        [W1[c * D:(c + 1) * D, m * D:(m + 1) * D] for c in range(3) for m in range(4)],
        axis=1).astype(bf)
    w2b = np.concatenate([W2[m * D:(m + 1) * D, :] for m in range(4)], axis=1).astype(bf)

    b1 = f["b1"].astype(np.float32)
    b2 = f["b2"].astype(np.float32)
    gm = f["ln_gamma"].astype(np.float32)
    bt = f["ln_beta"].astype(np.float32)
    trivial = (not b1.any()) and (not b2.any()) and (not bt.any()) and bool(np.all(gm == 1.0))

    in_maps = []
    for i in range(N_CORES):
        lo, hi = i * EC, (i + 1) * EC
        m = {
            "nodes": nodes,
            "edges": np.ascontiguousarray(e_bf[lo:hi]),
            "idx": np.ascontiguousarray(idx_pad[lo:hi]),
            "w1": w1b,
            "w2": w2b,
        }
        if not trivial:
            m["b1d"] = np.ascontiguousarray(b1.reshape(4, D).T.astype(np.float32))
            m["b2d"] = np.broadcast_to(b2, (D, D)).copy()
            m["gmd"] = np.broadcast_to(gm, (D, D)).copy()
            m["btd"] = np.broadcast_to(bt, (D, D)).copy()
        in_maps.append(m)
    return in_maps, trivial, E


def kernel(**inputs) -> np.ndarray:
    in_maps, trivial, E = _prep(inputs)
    nc = _build_program(trivial)
    res = run_bass_kernel_spmd(nc, in_maps, core_ids=list(range(N_CORES)))
    out = np.concatenate([res.results[i]["out"] for i in range(N_CORES)], axis=0)
    return np.ascontiguousarray(out[:E])
